# revision 6
# baseline (speedup 1.0000x reference)
"""MoE layer (top-2 of 8 experts, selection shared across tokens) on 8 TRN2 cores.

Math (faithful to the reference):
    gates = softmax(x @ W_gate + b_gate)          [N, 8]
    idx0  = top-2 expert indices of token 0       [2]
    s     = per-token top-2 gate VALUES (desc)    [N, 2]
    out   = s0 * (x @ W[A] + b[A]) + s1 * (x @ W[B] + b[B])

Strategy: gating + top-2 is 0.2% of the FLOPs -> computed on host.  The two
active expert matmuls (275 GFLOP) are data-parallel sharded over tokens across
8 cores; expert weights are replicated.  Matmuls run in fp16 (values are small,
so fp16 range is safe and its 10-bit mantissa keeps rel-err ~3e-4),
accumulating fp32 in PSUM.  The bias + score-weighted bias term
(s0*bA + s1*bB = scores @ b_sel) is rank-2 and added on the host, so the
device epilogue is only 2 DVE ops and the output DMAs bf16.

Both x (8 MB) and W (16 MB) are SBUF-resident (192 KB of the 208 KB/partition),
loaded once in a handful of large batched DMAs ordered exactly by first-use
time across the two HWDGE rings (sync carries k-chunks 0-7 + expert A, scalar
k-chunks 8-15 + expert B).  The output column space is split [256, 512, 512,
512, 256]: the narrow first block cuts the DMA-paced warmup (only 3 MB must
land before the first psum group completes), the narrow last block shortens
the epilogue+writeback tail.  Warm matmul cadence is the PE roofline
(N/2.4GHz + ~2.5ns), so the kernel sits within ~5% of the fp16 compute bound.
"""

import functools

import numpy as np

import concourse.bass as bass
import concourse.mybir as mybir
import concourse.tile as tile
from concourse import bacc
from concourse.bass_utils import run_bass_kernel_spmd

N_CORES = 8
N, D_IN, D_HID = 16384, 2048, 2048
NT = N // N_CORES            # tokens per core
KP = 128                     # contraction chunk = partition dim
KCH = D_IN // KP             # 16 K-chunks
MG = NT // 128               # 16 m-groups (psum partition tiles) per core
TQ0 = 256                    # tokens pre-loaded before the first matmul

# output column blocks: narrow first (fast DMA-paced ramp), narrow last
# (short tail).  N=256 is the narrowest block where the matmul stream
# (256/2.4GHz = 107ns) still hides the FWL weight load (~97ns).
BLOCKS = [(0, 256), (256, 512), (768, 512), (1280, 512), (1792, 256)]

F32 = mybir.dt.float32
BF16 = mybir.dt.bfloat16
FP16 = mybir.dt.float16

W_DT = FP16
X_DT = FP16

# Filled by test harness inspection: last BassKernelResults from a run.
LAST_RESULT = None


@functools.lru_cache(maxsize=1)
def _build():
    nc = bacc.Bacc("TRN2", target_bir_lowering=False, debug=False)
    # Host pre-arranges both streams partition-major so every DMA is a wide
    # strided copy with >=512B contiguous lines:
    #   xt3[p, k, t] = x[t, 128k+p]          (fp16)
    #   w?3[p, k, c] = W_expert[128k+p, c]   (fp16)
    xt3 = nc.dram_tensor("xt3", [KP, KCH, NT], X_DT, kind="ExternalInput")
    wa3 = nc.dram_tensor("wa3", [KP, KCH, D_HID], W_DT, kind="ExternalInput")
    wb3 = nc.dram_tensor("wb3", [KP, KCH, D_HID], W_DT, kind="ExternalInput")
    # per-token scores pre-arranged on host, partition-major:
    # sC[p, m, s] = top2_score[m*128 + p, s]
    sC = nc.dram_tensor("sC", [KP, MG, 2], F32, kind="ExternalInput")
    out = nc.dram_tensor("out", [NT, D_HID], BF16, kind="ExternalOutput")

    MULT = mybir.AluOpType.mult
    ADD = mybir.AluOpType.add

    with tile.TileContext(nc) as tc:
        with (
            tc.tile_pool(name="cst", bufs=1) as cst,
            tc.tile_pool(name="ep", bufs=2) as ep,
            tc.tile_pool(name="ps", bufs=3, space=bass.MemorySpace.PSUM) as ps,
        ):
            # x and both expert weight matrices live in SBUF for the whole
            # kernel (64 + 128 KB/partition).
            xr = cst.tile([KP, KCH, NT], X_DT)
            wr = (cst.tile([KP, KCH, D_HID], W_DT, name="wrA"),
                  cst.tile([KP, KCH, D_HID], W_DT, name="wrB"))

            # sync + scalar are the two fast HWDGE rings; each ring executes
            # its DMAs FIFO, so the issue order below IS the arrival order.
            # Constraints learned from traces: (a) the Tile scheduler has
            # only 8 DMA-completion sem lanes, so keep the opening wave to
            # <=8 transfers or later issues stall on lane reuse; (b) small
            # transfers are descriptor-bound early on, so keep every piece
            # >=0.5MB.  The opening wave lands x(tokens 0:128) and the two
            # experts' first-block W halves CONCURRENTLY on the two rings,
            # then x streams ahead of the narrow first block's ~3.5us/group
            # consumption while the remaining W blocks follow in use order.
            b0 = slice(0, BLOCKS[0][1])
            nc.sync.dma_start(xr[:, :, 0:128], xt3[:, :, 0:128])
            nc.scalar.dma_start(wr[0][:, 0:8, b0], wa3[:, 0:8, b0])
            nc.sync.dma_start(wr[1][:, 0:8, b0], wb3[:, 0:8, b0])
            nc.scalar.dma_start(wr[0][:, 8:16, b0], wa3[:, 8:16, b0])
            nc.sync.dma_start(wr[1][:, 8:16, b0], wb3[:, 8:16, b0])
            nc.sync.dma_start(xr[:, :, 128:TQ0], xt3[:, :, 128:TQ0])
            # scores ride the (otherwise idle until the first output) SWDGE
            # queue; needed only by the first epilogue ~17us in.
            sC_sb = cst.tile([KP, MG, 2], F32)
            nc.gpsimd.dma_start(sC_sb[:], sC[:])

            def both(fn):
                fn(nc.sync, slice(0, 8), wr[1], wb3)
                fn(nc.scalar, slice(8, 16), wr[0], wa3)

            # x remainder (k-halves split across the rings)
            for t0, t1 in ((TQ0, 1024), (1024, 2048)):
                both(lambda eng, ks, w, wd, t0=t0, t1=t1:
                     eng.dma_start(xr[:, ks, t0:t1], xt3[:, ks, t0:t1]))
            # W for the remaining blocks, in first-use order
            for c0, nb in BLOCKS[1:]:
                both(lambda eng, ks, w, wd, sl=slice(c0, c0 + nb):
                     eng.dma_start(w[:, :, sl], wd[:, :, sl]))

            for bi, (c0, nb) in enumerate(BLOCKS):
                csl = slice(c0, c0 + nb)
                last_blk = bi == len(BLOCKS) - 1
                for mg in range(MG):
                    # psum tiles stay full-bank (512 fp32) and narrow blocks
                    # use a 256-col slice: a matmul's start=True clears the
                    # whole bank, so two accumulating tiles must never share
                    # one.
                    pa = ps.tile([128, 512], F32, tag="pa")
                    pb = ps.tile([128, 512], F32, tag="pb")
                    for k in range(KCH):
                        xk = xr[:, k, bass.ts(mg, 128)]
                        nc.tensor.matmul(
                            pa[:, 0:nb], xk, wr[0][:, k, csl],
                            start=(k == 0), stop=(k == KCH - 1),
                        )
                        nc.tensor.matmul(
                            pb[:, 0:nb], xk, wr[1][:, k, csl],
                            start=(k == 0), stop=(k == KCH - 1),
                        )
                    s0 = sC_sb[:, mg, 0:1]
                    s1 = sC_sb[:, mg, 1:2]
                    # epilogue on DVE: out = s0*pa + s1*pb (bias is host-side)
                    # (each op reads at most one PSUM input)
                    t1_ = ep.tile([128, 512], F32, tag="t1")
                    nc.vector.tensor_scalar_mul(t1_[:, 0:nb], pa[:, 0:nb], s0)
                    o = ep.tile([128, 512], BF16, tag="o")
                    nc.vector.scalar_tensor_tensor(
                        o[:, 0:nb], pb[:, 0:nb], s1, t1_[:, 0:nb],
                        op0=MULT, op1=ADD,
                    )
                    # outputs ride SWDGE; in the last block the HWDGE rings
                    # are drained, so spread there for a short tail.
                    if last_blk:
                        eng = (nc.sync, nc.scalar, nc.gpsimd)[mg % 3]
                    else:
                        eng = nc.gpsimd
                    eng.dma_start(out[bass.ts(mg, 128), csl], o[:, 0:nb])

    nc.compile()
    return nc


def _host_gating(x, W_gate, b_gate):
    logits = x @ W_gate + b_gate                       # [N, 8] fp32
    m = logits.max(axis=1, keepdims=True)
    e = np.exp(logits - m)
    gates = e / e.sum(axis=1, keepdims=True)
    idx0 = np.argsort(-gates[0], kind="stable")[:2]    # token-0 top-2 experts
    scores = -np.sort(-gates, axis=1)[:, :2]           # per-token top-2 values
    return idx0, np.ascontiguousarray(scores)


def kernel(x, W_experts, b_experts, W_gate, b_gate):
    global LAST_RESULT
    x = np.ascontiguousarray(np.asarray(x, dtype=np.float32))
    W_experts = np.asarray(W_experts, dtype=np.float32)
    b_experts = np.asarray(b_experts, dtype=np.float32)
    W_gate = np.asarray(W_gate, dtype=np.float32)
    b_gate = np.asarray(b_gate, dtype=np.float32)

    idx0, scores = _host_gating(x, W_gate, b_gate)
    w_np_dt = mybir.dt.np(W_DT)
    x_np_dt = mybir.dt.np(X_DT)
    # wa3[p, k, c] = W[128k+p, c]
    wa3 = np.ascontiguousarray(
        W_experts[idx0[0]].reshape(KCH, KP, D_HID).transpose(1, 0, 2)
    ).astype(w_np_dt)
    wb3 = np.ascontiguousarray(
        W_experts[idx0[1]].reshape(KCH, KP, D_HID).transpose(1, 0, 2)
    ).astype(w_np_dt)

    xT_full = x.astype(x_np_dt).T                      # [D_IN, N]

    nc = _build()
    in_maps = []
    for c in range(N_CORES):
        sl = slice(c * NT, (c + 1) * NT)
        in_maps.append(
            {
                # xt3[p, k, t] = x[t0+t, 128k+p]
                "xt3": np.ascontiguousarray(
                    xT_full[:, sl].reshape(KCH, KP, NT).transpose(1, 0, 2)
                ),
                "wa3": wa3,
                "wb3": wb3,
                "sC": np.ascontiguousarray(
                    scores[sl].reshape(MG, 128, 2).transpose(1, 0, 2)
                ),
            }
        )

    res = run_bass_kernel_spmd(nc, in_maps, list(range(N_CORES)))
    LAST_RESULT = res
    out = np.concatenate(
        [np.asarray(r["out"]) for r in res.results], axis=0
    ).astype(np.float32)
    # rank-2 bias term folded out of the device epilogue:
    # s0*bA + s1*bB = scores @ b_sel
    out += scores.astype(np.float32) @ b_experts[idx0].astype(np.float32)
    return out


# revision 8
# speedup vs baseline: 1.0268x; 1.0268x over previous
"""MoE layer (top-2 of 8 experts, selection shared across tokens) on 8 TRN2 cores.

Math (faithful to the reference):
    gates = softmax(x @ W_gate + b_gate)          [N, 8]
    idx0  = top-2 expert indices of token 0       [2]
    s     = per-token top-2 gate VALUES (desc)    [N, 2]
    out   = s0 * (x @ W[A] + b[A]) + s1 * (x @ W[B] + b[B])

Strategy: gating + top-2 is 0.2% of the FLOPs -> computed on host.  The two
active expert matmuls (275 GFLOP) are data-parallel sharded over tokens across
8 cores; expert weights are replicated.  Matmuls run in fp16 (values are small,
so fp16 range is safe and its 10-bit mantissa keeps rel-err ~3e-4),
accumulating fp32 in PSUM.  The bias + score-weighted bias term
(s0*bA + s1*bB = scores @ b_sel) is rank-2 and added on the host, so the
device epilogue is only 2 DVE ops and the output DMAs bf16.

Both x (8 MB) and W (16 MB) are SBUF-resident (192 KB of the 208 KB/partition),
loaded once in a handful of large batched DMAs ordered exactly by first-use
time across the two HWDGE rings (sync carries k-chunks 0-7 + expert A, scalar
k-chunks 8-15 + expert B).  The output column space is split [256, 512, 512,
512, 256]: the narrow first block cuts the DMA-paced warmup (only 3 MB must
land before the first psum group completes), the narrow last block shortens
the epilogue+writeback tail.  Warm matmul cadence is the PE roofline
(N/2.4GHz + ~2.5ns), so the kernel sits within ~5% of the fp16 compute bound.
"""

import functools

import numpy as np

import concourse.bass as bass
import concourse.mybir as mybir
import concourse.tile as tile
from concourse import bacc
from concourse.bass_utils import run_bass_kernel_spmd

N_CORES = 8
N, D_IN, D_HID = 16384, 2048, 2048
NT = N // N_CORES            # tokens per core
KP = 128                     # contraction chunk = partition dim
KCH = D_IN // KP             # 16 K-chunks
MG = NT // 128               # 16 m-groups (psum partition tiles) per core
TQ0 = 256                    # tokens pre-loaded before the first matmul

# output column blocks: uniform 512 (one fp32 PSUM bank).  Narrow first/last
# blocks were tried and measured slower: the wide first block's DMA-paced
# psum group absorbs the HAM cold-clock phase for free, and narrow blocks
# inflate matmul count.
BLOCKS = [(0, 512), (512, 512), (1024, 512), (1536, 512)]

F32 = mybir.dt.float32
BF16 = mybir.dt.bfloat16
FP16 = mybir.dt.float16

W_DT = FP16
X_DT = FP16

# Filled by test harness inspection: last BassKernelResults from a run.
LAST_RESULT = None


@functools.lru_cache(maxsize=1)
def _build():
    nc = bacc.Bacc("TRN2", target_bir_lowering=False, debug=False)
    # Host pre-arranges both streams partition-major so every DMA is a wide
    # strided copy with >=512B contiguous lines:
    #   xt3[p, k, t] = x[t, 128k+p]          (fp16)
    #   w?3[p, k, c] = W_expert[128k+p, c]   (fp16)
    xt3 = nc.dram_tensor("xt3", [KP, KCH, NT], X_DT, kind="ExternalInput")
    wa3 = nc.dram_tensor("wa3", [KP, KCH, D_HID], W_DT, kind="ExternalInput")
    wb3 = nc.dram_tensor("wb3", [KP, KCH, D_HID], W_DT, kind="ExternalInput")
    # per-token scores pre-arranged on host, partition-major:
    # sC[p, m, s] = top2_score[m*128 + p, s]
    sC = nc.dram_tensor("sC", [KP, MG, 2], F32, kind="ExternalInput")
    out = nc.dram_tensor("out", [NT, D_HID], BF16, kind="ExternalOutput")

    MULT = mybir.AluOpType.mult
    ADD = mybir.AluOpType.add

    with tile.TileContext(nc) as tc:
        with (
            tc.tile_pool(name="cst", bufs=1) as cst,
            tc.tile_pool(name="ep", bufs=2) as ep,
            tc.tile_pool(name="ps", bufs=3, space=bass.MemorySpace.PSUM) as ps,
        ):
            # x and both expert weight matrices live in SBUF for the whole
            # kernel (64 + 128 KB/partition).
            xr = cst.tile([KP, KCH, NT], X_DT)
            wr = (cst.tile([KP, KCH, D_HID], W_DT, name="wrA"),
                  cst.tile([KP, KCH, D_HID], W_DT, name="wrB"))

            # sync + scalar are the two fast HWDGE rings; each ring executes
            # its DMAs FIFO, so the issue order below IS the arrival order.
            # Trace-derived constraints: (a) only 8 DMA-completion sem lanes
            # exist, so an opening wave of >8 transfers stalls later issues
            # on lane reuse; (b) pieces must keep >=512B contiguous lines
            # (smaller lines hit the SDMA read-modify-write path) and be
            # >=0.5MB or they are descriptor-bound.  The first matmul needs
            # x(k0), wA(k0), wB(k0): x + expert B ride sync, expert A rides
            # scalar, so the three land concurrently ~12us in, and the
            # DMA-paced first psum group absorbs the HAM cold-clock window.
            b0 = slice(0, 512)
            nc.sync.dma_start(xr[:, 0:8, 0:TQ0], xt3[:, 0:8, 0:TQ0])
            nc.scalar.dma_start(wr[0][:, 0:4, b0], wa3[:, 0:4, b0])
            nc.sync.dma_start(wr[1][:, 0:4, b0], wb3[:, 0:4, b0])
            nc.scalar.dma_start(wr[0][:, 4:8, b0], wa3[:, 4:8, b0])
            nc.sync.dma_start(wr[1][:, 4:8, b0], wb3[:, 4:8, b0])
            nc.scalar.dma_start(wr[0][:, 8:12, b0], wa3[:, 8:12, b0])
            nc.sync.dma_start(xr[:, 8:16, 0:TQ0], xt3[:, 8:16, 0:TQ0])
            nc.scalar.dma_start(wr[0][:, 12:16, b0], wa3[:, 12:16, b0])
            nc.sync.dma_start(wr[1][:, 8:12, b0], wb3[:, 8:12, b0])
            nc.sync.dma_start(wr[1][:, 12:16, b0], wb3[:, 12:16, b0])
            # scores ride the (otherwise idle until the first output) SWDGE
            # queue; needed only by the first epilogue ~19us in.
            sC_sb = cst.tile([KP, MG, 2], F32)
            nc.gpsimd.dma_start(sC_sb[:], sC[:])

            # x remainder then the remaining W blocks, in first-use order;
            # expert A stays on scalar, x + expert B on sync.
            for t0, t1 in ((TQ0, 1024), (1024, 2048)):
                nc.sync.dma_start(xr[:, 0:8, t0:t1], xt3[:, 0:8, t0:t1])
                nc.scalar.dma_start(xr[:, 8:16, t0:t1], xt3[:, 8:16, t0:t1])
            for c0, nb in BLOCKS[1:]:
                sl = slice(c0, c0 + nb)
                nc.scalar.dma_start(wr[0][:, :, sl], wa3[:, :, sl])
                nc.sync.dma_start(wr[1][:, :, sl], wb3[:, :, sl])

            for bi, (c0, nb) in enumerate(BLOCKS):
                csl = slice(c0, c0 + nb)
                last_blk = bi == len(BLOCKS) - 1
                for mg in range(MG):
                    # psum tiles stay full-bank (512 fp32) and narrow blocks
                    # use a 256-col slice: a matmul's start=True clears the
                    # whole bank, so two accumulating tiles must never share
                    # one.
                    pa = ps.tile([128, 512], F32, tag="pa")
                    pb = ps.tile([128, 512], F32, tag="pb")
                    for k in range(KCH):
                        xk = xr[:, k, bass.ts(mg, 128)]
                        nc.tensor.matmul(
                            pa[:, 0:nb], xk, wr[0][:, k, csl],
                            start=(k == 0), stop=(k == KCH - 1),
                        )
                        nc.tensor.matmul(
                            pb[:, 0:nb], xk, wr[1][:, k, csl],
                            start=(k == 0), stop=(k == KCH - 1),
                        )
                    s0 = sC_sb[:, mg, 0:1]
                    s1 = sC_sb[:, mg, 1:2]
                    # epilogue on DVE: out = s0*pa + s1*pb (bias is host-side)
                    # (each op reads at most one PSUM input)
                    t1_ = ep.tile([128, 512], F32, tag="t1")
                    nc.vector.tensor_scalar_mul(t1_[:, 0:nb], pa[:, 0:nb], s0)
                    o = ep.tile([128, 512], BF16, tag="o")
                    nc.vector.scalar_tensor_tensor(
                        o[:, 0:nb], pb[:, 0:nb], s1, t1_[:, 0:nb],
                        op0=MULT, op1=ADD,
                    )
                    # outputs ride SWDGE; in the last block the HWDGE rings
                    # are drained, so spread there for a short tail.
                    if last_blk:
                        eng = (nc.sync, nc.scalar, nc.gpsimd)[mg % 3]
                    else:
                        eng = nc.gpsimd
                    eng.dma_start(out[bass.ts(mg, 128), csl], o[:, 0:nb])

    nc.compile()
    return nc


def _host_gating(x, W_gate, b_gate):
    logits = x @ W_gate + b_gate                       # [N, 8] fp32
    m = logits.max(axis=1, keepdims=True)
    e = np.exp(logits - m)
    gates = e / e.sum(axis=1, keepdims=True)
    idx0 = np.argsort(-gates[0], kind="stable")[:2]    # token-0 top-2 experts
    scores = -np.sort(-gates, axis=1)[:, :2]           # per-token top-2 values
    return idx0, np.ascontiguousarray(scores)


def kernel(x, W_experts, b_experts, W_gate, b_gate):
    global LAST_RESULT
    x = np.ascontiguousarray(np.asarray(x, dtype=np.float32))
    W_experts = np.asarray(W_experts, dtype=np.float32)
    b_experts = np.asarray(b_experts, dtype=np.float32)
    W_gate = np.asarray(W_gate, dtype=np.float32)
    b_gate = np.asarray(b_gate, dtype=np.float32)

    idx0, scores = _host_gating(x, W_gate, b_gate)
    w_np_dt = mybir.dt.np(W_DT)
    x_np_dt = mybir.dt.np(X_DT)
    # wa3[p, k, c] = W[128k+p, c]
    wa3 = np.ascontiguousarray(
        W_experts[idx0[0]].reshape(KCH, KP, D_HID).transpose(1, 0, 2)
    ).astype(w_np_dt)
    wb3 = np.ascontiguousarray(
        W_experts[idx0[1]].reshape(KCH, KP, D_HID).transpose(1, 0, 2)
    ).astype(w_np_dt)

    xT_full = x.astype(x_np_dt).T                      # [D_IN, N]

    nc = _build()
    in_maps = []
    for c in range(N_CORES):
        sl = slice(c * NT, (c + 1) * NT)
        in_maps.append(
            {
                # xt3[p, k, t] = x[t0+t, 128k+p]
                "xt3": np.ascontiguousarray(
                    xT_full[:, sl].reshape(KCH, KP, NT).transpose(1, 0, 2)
                ),
                "wa3": wa3,
                "wb3": wb3,
                "sC": np.ascontiguousarray(
                    scores[sl].reshape(MG, 128, 2).transpose(1, 0, 2)
                ),
            }
        )

    res = run_bass_kernel_spmd(nc, in_maps, list(range(N_CORES)))
    LAST_RESULT = res
    out = np.concatenate(
        [np.asarray(r["out"]) for r in res.results], axis=0
    ).astype(np.float32)
    # rank-2 bias term folded out of the device epilogue:
    # s0*bA + s1*bB = scores @ b_sel
    out += scores.astype(np.float32) @ b_experts[idx0].astype(np.float32)
    return out


# revision 9
# speedup vs baseline: 1.0393x; 1.0122x over previous
"""MoE layer (top-2 of 8 experts, selection shared across tokens) on 8 TRN2 cores.

Math (faithful to the reference):
    gates = softmax(x @ W_gate + b_gate)          [N, 8]
    idx0  = top-2 expert indices of token 0       [2]
    s     = per-token top-2 gate VALUES (desc)    [N, 2]
    out   = s0 * (x @ W[A] + b[A]) + s1 * (x @ W[B] + b[B])

Strategy: gating + top-2 is 0.2% of the FLOPs -> computed on host.  The two
active expert matmuls (275 GFLOP) are data-parallel sharded over tokens across
8 cores; expert weights are replicated.  Matmuls run in fp16 (values are small,
so fp16 range is safe and its 10-bit mantissa keeps rel-err ~3e-4),
accumulating fp32 in PSUM.  The bias + score-weighted bias term
(s0*bA + s1*bB = scores @ b_sel) is rank-2 and added on the host, so the
device epilogue is only 2 DVE ops and the output DMAs bf16.

DMA layout: x is SBUF-resident (8 MB fp16, loaded once in a handful of large
batched DMAs), W streams one 512-col block per expert per DMA (2 MB each,
triple-buffered).  Each HWDGE ring carries one x half first, then one
expert's W (sync: x k0:8 + expert A; scalar: x k8:16 + expert B) — keeping x
in front of W on both rings avoids mid-k-loop starvation and HAM
re-throttling.  The warm matmul cadence is the PE roofline (216 ns per N=512
fp16 matmul); the only overheads left are the fixed engine preamble, the
DMA-paced first psum group (which absorbs the HAM cold-clock window), and
the tail drain.
"""

import functools

import numpy as np

import concourse.bass as bass
import concourse.mybir as mybir
import concourse.tile as tile
from concourse import bacc
from concourse.bass_utils import run_bass_kernel_spmd

N_CORES = 8
N, D_IN, D_HID = 16384, 2048, 2048
NT = N // N_CORES            # tokens per core
KP = 128                     # contraction chunk = partition dim
KCH = D_IN // KP             # 16 K-chunks
NB = 512                     # output column block (1 PSUM bank of fp32)
NBLK = D_HID // NB           # 4 output blocks
MG = NT // 128               # 16 m-groups (psum partition tiles) per core
TQ0 = 256                    # tokens pre-loaded before the first matmul

F32 = mybir.dt.float32
BF16 = mybir.dt.bfloat16
FP16 = mybir.dt.float16

W_DT = FP16
X_DT = FP16

# Filled by test harness inspection: last BassKernelResults from a run.
LAST_RESULT = None


@functools.lru_cache(maxsize=1)
def _build():
    nc = bacc.Bacc("TRN2", target_bir_lowering=False, debug=False)
    # Host pre-arranges both streams partition-major so every DMA is a wide
    # strided copy with >=512B contiguous lines:
    #   xt3[p, k, t] = x[t, 128k+p]          (fp16)
    #   w?3[p, k, c] = W_expert[128k+p, c]   (fp16)
    xt3 = nc.dram_tensor("xt3", [KP, KCH, NT], X_DT, kind="ExternalInput")
    wa3 = nc.dram_tensor("wa3", [KP, KCH, D_HID], W_DT, kind="ExternalInput")
    wb3 = nc.dram_tensor("wb3", [KP, KCH, D_HID], W_DT, kind="ExternalInput")
    # per-token scores pre-arranged on host, partition-major:
    # sC[p, m, s] = top2_score[m*128 + p, s]
    sC = nc.dram_tensor("sC", [KP, MG, 2], F32, kind="ExternalInput")
    out = nc.dram_tensor("out", [NT, D_HID], BF16, kind="ExternalOutput")

    MULT = mybir.AluOpType.mult
    ADD = mybir.AluOpType.add

    with tile.TileContext(nc) as tc:
        with (
            tc.tile_pool(name="cst", bufs=1) as cst,
            tc.tile_pool(name="wp", bufs=3) as wp,
            tc.tile_pool(name="ep", bufs=2) as ep,
            tc.tile_pool(name="ps", bufs=3, space=bass.MemorySpace.PSUM) as ps,
        ):
            # x lives in SBUF for the whole kernel (64 KB/partition).
            xr = cst.tile([KP, KCH, NT], X_DT)
            # sync + scalar are the two fast HWDGE rings; each ring executes
            # its DMAs FIFO, so the issue order below IS the arrival order.
            # Critical path first: the q0 token slice + the first W k-groups.
            nc.sync.dma_start(xr[:, 0:8, 0:TQ0], xt3[:, 0:8, 0:TQ0])
            nc.scalar.dma_start(xr[:, 8:16, 0:TQ0], xt3[:, 8:16, 0:TQ0])

            # scores ride the (otherwise idle until the first output) SWDGE
            # queue; needed only by the first epilogue ~20us in.
            sC_sb = cst.tile([KP, MG, 2], F32)
            nc.gpsimd.dma_start(sC_sb[:], sC[:])

            w_t = {}

            def load_w_block(nb):
                """Issue the W loads for column block nb (both experts)."""
                ta = wp.tile([KP, KCH, NB], W_DT, tag="wA")
                tb = wp.tile([KP, KCH, NB], W_DT, tag="wB")
                sl = bass.ts(nb, NB)
                if nb == 0:
                    # fine-grained k-groups so the first k-loop can start as
                    # soon as the first 4 chunks land
                    for g in range(4):
                        ks = slice(4 * g, 4 * g + 4)
                        nc.sync.dma_start(ta[:, ks, :], wa3[:, ks, sl])
                        nc.scalar.dma_start(tb[:, ks, :], wb3[:, ks, sl])
                else:
                    nc.sync.dma_start(ta[:], wa3[:, :, sl])
                    nc.scalar.dma_start(tb[:], wb3[:, :, sl])
                w_t[nb] = (ta, tb)

            load_w_block(0)
            # x remainder in two pieces per ring: tokens 256:1024 are needed
            # from m-group 2 (~25us), the rest much later.
            nc.sync.dma_start(xr[:, 0:8, TQ0:1024], xt3[:, 0:8, TQ0:1024])
            nc.scalar.dma_start(xr[:, 8:16, TQ0:1024], xt3[:, 8:16, TQ0:1024])
            nc.sync.dma_start(xr[:, 0:8, 1024:NT], xt3[:, 0:8, 1024:NT])
            nc.scalar.dma_start(xr[:, 8:16, 1024:NT], xt3[:, 8:16, 1024:NT])
            load_w_block(1)

            for nb in range(NBLK):
                nb_sl = bass.ts(nb, NB)
                if nb + 2 < NBLK:
                    # prefetch two blocks ahead (bufs=3); the dma_start just
                    # parks the issuing ring until its WAR semaphore clears.
                    load_w_block(nb + 2)
                ta, tb = w_t.pop(nb)
                for mg in range(MG):
                    pa = ps.tile([128, NB], F32, tag="pa")
                    pb = ps.tile([128, NB], F32, tag="pb")
                    for k in range(KCH):
                        xk = xr[:, k, bass.ts(mg, 128)]
                        nc.tensor.matmul(
                            pa[:], xk, ta[:, k, :],
                            start=(k == 0), stop=(k == KCH - 1),
                        )
                        nc.tensor.matmul(
                            pb[:], xk, tb[:, k, :],
                            start=(k == 0), stop=(k == KCH - 1),
                        )
                    s0 = sC_sb[:, mg, 0:1]
                    s1 = sC_sb[:, mg, 1:2]
                    # epilogue on DVE: out = s0*pa + s1*pb (bias is host-side)
                    # (each op reads at most one PSUM input)
                    t1 = ep.tile([128, NB], F32, tag="t1")
                    nc.vector.tensor_scalar_mul(t1[:], pa[:], s0)
                    o = ep.tile([128, NB], BF16, tag="o")
                    nc.vector.scalar_tensor_tensor(
                        o[:], pb[:], s1, t1[:], op0=MULT, op1=ADD
                    )
                    # outputs ride SWDGE; in the last block the HWDGE rings
                    # are drained, so spread there for a short tail.
                    if nb == NBLK - 1:
                        eng = (nc.sync, nc.scalar, nc.gpsimd)[mg % 3]
                    else:
                        eng = nc.gpsimd
                    eng.dma_start(out[bass.ts(mg, 128), nb_sl], o[:])

    nc.compile()
    return nc


def _host_gating(x, W_gate, b_gate):
    logits = x @ W_gate + b_gate                       # [N, 8] fp32
    m = logits.max(axis=1, keepdims=True)
    e = np.exp(logits - m)
    gates = e / e.sum(axis=1, keepdims=True)
    idx0 = np.argsort(-gates[0], kind="stable")[:2]    # token-0 top-2 experts
    scores = -np.sort(-gates, axis=1)[:, :2]           # per-token top-2 values
    return idx0, np.ascontiguousarray(scores)


def kernel(x, W_experts, b_experts, W_gate, b_gate):
    global LAST_RESULT
    x = np.ascontiguousarray(np.asarray(x, dtype=np.float32))
    W_experts = np.asarray(W_experts, dtype=np.float32)
    b_experts = np.asarray(b_experts, dtype=np.float32)
    W_gate = np.asarray(W_gate, dtype=np.float32)
    b_gate = np.asarray(b_gate, dtype=np.float32)

    idx0, scores = _host_gating(x, W_gate, b_gate)
    w_np_dt = mybir.dt.np(W_DT)
    x_np_dt = mybir.dt.np(X_DT)
    # wa3[p, k, c] = W[128k+p, c]
    wa3 = np.ascontiguousarray(
        W_experts[idx0[0]].reshape(KCH, KP, D_HID).transpose(1, 0, 2)
    ).astype(w_np_dt)
    wb3 = np.ascontiguousarray(
        W_experts[idx0[1]].reshape(KCH, KP, D_HID).transpose(1, 0, 2)
    ).astype(w_np_dt)

    xT_full = x.astype(x_np_dt).T                      # [D_IN, N]

    nc = _build()
    in_maps = []
    for c in range(N_CORES):
        sl = slice(c * NT, (c + 1) * NT)
        in_maps.append(
            {
                # xt3[p, k, t] = x[t0+t, 128k+p]
                "xt3": np.ascontiguousarray(
                    xT_full[:, sl].reshape(KCH, KP, NT).transpose(1, 0, 2)
                ),
                "wa3": wa3,
                "wb3": wb3,
                "sC": np.ascontiguousarray(
                    scores[sl].reshape(MG, 128, 2).transpose(1, 0, 2)
                ),
            }
        )

    res = run_bass_kernel_spmd(nc, in_maps, list(range(N_CORES)))
    LAST_RESULT = res
    out = np.concatenate(
        [np.asarray(r["out"]) for r in res.results], axis=0
    ).astype(np.float32)
    # rank-2 bias term folded out of the device epilogue:
    # s0*bA + s1*bB = scores @ b_sel
    out += scores.astype(np.float32) @ b_experts[idx0].astype(np.float32)
    return out
